# revision 5
# baseline (speedup 1.0000x reference)
"""CASVDDenseMul fused kernel for 8 Trainium2 NeuronCores.

Reference computation (fp32):
    chi = sigmoid(context @ W + B)          # [B, R]
    t   = (inputs @ U) * (S * chi)          # [B, R]
    out = relu(t @ V.T + 2*bias)            # [B, UNITS]

Sharding: data-parallel over batch; each of the 8 cores handles 512 rows.
All factor weights (U, S, V, W, B, bias) are replicated.

Design notes (v2 -- wire-saturation-first):
  - Everything travels as bf16 (PE runs one element/cell/cycle for any
    dtype; rel-err ~3.5e-3 vs the 2e-2 gate; fp8 anywhere measured over
    the gate). S folded into U's columns on the host. 13.4MB/core total
    HBM traffic = ~35us of wire at the ~383GB/s the two HWDGE rings
    sustain; PE work is ~29us -- the kernel is wire-bound, so the whole
    game is keeping the HBM wire busy 100% of the span.
  - v1 streamed x fully before VT and ran all of mm2 after the stream:
    the wire idled 5.4us (all inputs landed, no outputs ready yet) and
    mm2's 13.8us ran serial after it.  v2 reorders the stream
    [U+xA | wctx | VT | xB] and the PE program
    [chi, mm1-A, t'A, mm2-A, mm1-B, t'B, mm2-B]: mm2 on batch
    sub-block A runs inside xB's streaming window, so after the last
    input byte only mm1-B's tail + mm2-B remain on the PE while the
    16 output waves (staged in SBUF, queued FIFO behind the inputs on
    the same two rings) drain the wire without a gap.
  - Ring order == consumption order, ~4.6MB per ring balanced so both
    rings finish inputs together (outputs on a ring drain strictly
    after that ring's inputs -- HWDGE rings are FIFO -- so early waves
    can't steal bandwidth from the xB stream the PE is waiting on).
  - mm2 waves: two single-bank [128,512] PSUM tiles per 1024-unit wave
    from a 5-deep pool (bank reuse ~2.5 waves behind the matmuls,
    beyond the drain->evac->reuse latency chain). 512-wide matmuls,
    rank pairs share one LDWEIGHTS, evacuation split ACT(h0)/DVE(h1).
  - PSUM note: start=True clears has_written BANK-wide, so only the
    first matmul into a shared bank carries start=True.
"""

import numpy as np
import ml_dtypes

from concourse import bacc, mybir
from concourse import tile
from concourse.bass_utils import run_bass_kernel_spmd

N_CORES = 8
B_SZ, N_IN, N_CTX, UNITS, RANK = 4096, 4096, 512, 4096, 256
BS = B_SZ // N_CORES   # 512 batch rows per core

P = 128
KC_IN = N_IN // P      # 32 contraction chunks for x @ U
KC_CTX = N_CTX // P    # 4  contraction chunks for ctx @ W
RT = RANK // P         # 2  rank tiles
NQ = 4                 # U/x stream pieces
KPQ = KC_IN // NQ      # 8 chunks per piece
NSB = 2                # batch sub-blocks
BSB = BS // NSB        # 256 batch cols per sub-block
NBT = BSB // P         # 2 batch tiles (128) per sub-block
NW = 4                 # VT pieces / unit waves (1024 units each)
WU = UNITS // NW       # 1024 units per wave

BF16 = mybir.dt.bfloat16
FP32 = mybir.dt.float32
FP32R = mybir.dt.float32r

bf16 = ml_dtypes.bfloat16


def _build_nc(use_b, use_bias):
    nc = bacc.Bacc("TRN2", target_bir_lowering=False, debug=False, enable_asserts=False)

    wctx = nc.declare_dram_parameter("wctx", [P, KC_CTX * (RANK + BS)], BF16, isOutput=False)
    u8 = nc.declare_dram_parameter("u8", [NQ, P, KPQ * RANK], BF16, isOutput=False)
    xg = nc.declare_dram_parameter("xg", [NSB, NQ, P, KPQ * BSB], BF16, isOutput=False)
    vt4 = nc.declare_dram_parameter("vt4", [NW, P, RT * WU], BF16, isOutput=False)
    if use_b:
        bvec = nc.declare_dram_parameter("bvec", [P, RT], FP32, isOutput=False)
    if use_bias:
        brow = nc.declare_dram_parameter("brow", [1, P + UNITS], FP32R, isOutput=False)
    out_d = nc.declare_dram_parameter("out_d", [BS, UNITS], BF16, isOutput=True)

    with tile.TileContext(nc) as tc:
        with (
            tc.tile_pool(name="small", bufs=1) as small,
            tc.tile_pool(name="stream", bufs=1) as stream,
            tc.tile_pool(name="acts", bufs=1) as acts,
            tc.tile_pool(name="ostage", bufs=16) as ostage,
            tc.tile_pool(name="pt", bufs=1, space="PSUM") as pt,
            tc.tile_pool(name="pout", bufs=6, space="PSUM") as pout,
        ):
            # ---- SBUF tiles ----
            wctx_sb = small.tile([P, KC_CTX * (RANK + BS)], BF16, tag="wctx")
            u_sb = small.tile([P, NQ, KPQ * RANK], BF16, tag="u")
            x_sb = [[stream.tile([P, KPQ * BSB], BF16, tag=f"x{s}{g}", name=f"x{s}{g}")
                     for g in range(NQ)] for s in range(NSB)]
            vt_sb = small.tile([P, NW, RT * WU], BF16, tag="vt")
            if use_b:
                bvec_sb = small.tile([P, RT], FP32, tag="bvec")
            if use_bias:
                brow_sb = small.tile([1, P + UNITS], FP32R, tag="brow")
            s_chi = acts.tile([P, RT, BS], FP32, tag="schi")
            t_sb = [acts.tile([P, RT, BSB], BF16, tag=f"tsb{s}", name=f"tsb{s}")
                    for s in range(NSB)]
            junk = acts.tile([P, P], BF16, tag="junk")

            # ---- DMA issue queues.  Per-ring order == consumption
            # order; the two HWDGE rings are FIFO, so placing VT before
            # xB and all outputs after all inputs enforces the phase
            # plan at the wire level.  ~4.6MB per ring, balanced.
            HB = (KC_CTX // 2) * (RANK + BS)   # wctx half (cols)
            nc.sync.dma_start(u_sb[:, 0, :], u8[0])                    # A1
            nc.scalar.dma_start(x_sb[0][0][:], xg[0, 0])               # B1
            nc.sync.dma_start(x_sb[0][1][:], xg[0, 1])                 # A2
            nc.scalar.dma_start(wctx_sb[:, HB:], wctx[:, HB:])         # B2
            if use_b:
                nc.sync.dma_start(bvec_sb[:], bvec[:])
            if use_bias:
                nc.scalar.dma_start(brow_sb[:], brow[:])
            nc.sync.dma_start(wctx_sb[:, :HB], wctx[:, :HB])           # A3
            nc.scalar.dma_start(u_sb[:, 1, :], u8[1])                  # B3
            nc.sync.dma_start(u_sb[:, 2, :], u8[2])                    # A4
            nc.scalar.dma_start(x_sb[0][2][:], xg[0, 2])               # B4
            nc.sync.dma_start(x_sb[0][3][:], xg[0, 3])                 # A5
            nc.scalar.dma_start(u_sb[:, 3, :], u8[3])                  # B5
            nc.sync.dma_start(vt_sb[:, 0, :], vt4[0])                  # A6
            nc.scalar.dma_start(vt_sb[:, 1, :], vt4[1])                # B6
            nc.sync.dma_start(vt_sb[:, 2, :], vt4[2])                  # A7
            nc.scalar.dma_start(vt_sb[:, 3, :], vt4[3])                # B7
            nc.sync.dma_start(x_sb[1][0][:], xg[1, 0])                 # A8
            nc.scalar.dma_start(x_sb[1][1][:], xg[1, 1])               # B8
            nc.sync.dma_start(x_sb[1][2][:], xg[1, 2])                 # A9
            nc.scalar.dma_start(x_sb[1][3][:], xg[1, 3])               # B9

            psum_t = [pt.tile([P, RT * BSB], FP32, tag=f"pt{s}", name=f"pt{s}")
                      for s in range(NSB)]

            # ---- PE warm-up: keep the HAM activity window busy from t=0
            # so the clock gate lifts to 2.4 GHz before the real stream.
            # Targets psum_t[0], whose contents mm1-A's start=True clears.
            nc.gpsimd.memset(junk[:], 0.0)
            for _ in range(16):
                nc.tensor.matmul(
                    psum_t[0][:, :P], junk[:], junk[:],
                    start=True, stop=True, skip_group_check=True,
                )

            # ---- chi' = sigmoid(W.T @ ctxT + B)  (S folded into U) ----
            # chi psum tiles come from the pout pool (same [P,512] fp32
            # shape): chi-rt0/rt1 get their OWN banks, so the two matmul
            # groups + sigmoids pipeline instead of serializing on one
            # bank -- keeping chi (and thus t' and mm2-A) off the
            # critical path.  The banks recycle into the wave rotation.
            for rt in range(RT):
                psum_chi = pout.tile([P, BS], FP32, tag="po", name=f"pchi{rt}")
                for k in range(KC_CTX):
                    base = k * (RANK + BS)
                    nc.tensor.matmul(
                        psum_chi[:],
                        wctx_sb[:, base + rt * P: base + (rt + 1) * P],
                        wctx_sb[:, base + RANK: base + RANK + BS],
                        start=(k == 0), stop=(k == KC_CTX - 1),
                        skip_group_check=True,
                    )
                nc.scalar.activation(
                    s_chi[:, rt, :], psum_chi[:],
                    mybir.ActivationFunctionType.Sigmoid,
                    bias=(bvec_sb[:, rt:rt + 1] if use_b else 0.0), scale=1.0,
                )

            def emit_mm1_piece(s, q):
                # psum_t[s][:, rt*BSB:(rt+1)*BSB] += U'_k.T @ x_k, k in piece q.
                # Both rank-half groups share ONE psum bank; start=True clears
                # has_written BANK-wide, so only the very first matmul carries
                # it (the rt1 group's k=0 lands on cleared bits and start=False
                # already overwrites).
                for j in range(KPQ):
                    k = q * KPQ + j
                    for rt in range(RT):
                        nc.tensor.matmul(
                            psum_t[s][:, rt * BSB:(rt + 1) * BSB],
                            u_sb[:, q, j * RANK + rt * P: j * RANK + (rt + 1) * P],
                            x_sb[s][q][:, j * BSB:(j + 1) * BSB],
                            start=(k == 0 and rt == 0),
                            stop=(k == KC_IN - 1),
                            skip_group_check=True,
                        )

            def emit_tprime(s):
                for rt in range(RT):
                    nc.vector.tensor_mul(
                        t_sb[s][:, rt, :],
                        psum_t[s][:, rt * BSB:(rt + 1) * BSB],
                        s_chi[:, rt, s * BSB:(s + 1) * BSB],
                    )

            def emit_mm2_wave(s, w, bt, widx):
                # out[bt-rows, wave-units] = t'.T @ VT (+ 2*bias), relu, DMA.
                pw = [pout.tile([P, 512], FP32, tag="po", name=f"po{widx}h{h}")
                      for h in range(2)]
                for rt in range(RT):
                    for h in range(2):
                        nc.tensor.matmul(
                            pw[h][:],
                            t_sb[s][:, rt, bt * P:(bt + 1) * P],
                            vt_sb[:, w, rt * WU + h * 512: rt * WU + (h + 1) * 512],
                            start=(rt == 0),
                            stop=(rt == RT - 1 and not use_bias),
                            skip_group_check=True,
                        )
                if use_bias:
                    for h in range(2):
                        nc.tensor.matmul(
                            pw[h][:],
                            brow_sb[:, 0:P],
                            brow_sb[:, P + w * WU + h * 512: P + w * WU + (h + 1) * 512],
                            start=False, stop=True,
                            skip_group_check=True,
                        )
                o_sb = ostage.tile([P, WU], BF16, tag="osb")
                nc.scalar.activation(
                    o_sb[:, 0:512], pw[0][:],
                    mybir.ActivationFunctionType.Relu,
                )
                nc.vector.tensor_scalar(
                    o_sb[:, 512:WU], pw[1][:], 0.0, None,
                    op0=mybir.AluOpType.max,
                )
                rows = slice(s * BSB + bt * P, s * BSB + (bt + 1) * P)
                cols = slice(w * WU, (w + 1) * WU)
                eng = nc.scalar if widx % 2 == 0 else nc.sync
                eng.dma_start(out_d[rows, cols], o_sb[:])

            # PE program order: mm1-A rides the U/xA stream, mm2-A rides
            # VT's arrival (inside xB's streaming window), mm1-B's tail +
            # mm2-B are all that remain after the last input byte.
            widx = 0
            for q in range(NQ):
                emit_mm1_piece(0, q)
            emit_tprime(0)
            for w in range(NW):
                for bt in range(NBT):
                    emit_mm2_wave(0, w, bt, widx)
                    widx += 1
            for q in range(NQ):
                emit_mm1_piece(1, q)
            emit_tprime(1)
            for w in range(NW):
                for bt in range(NBT):
                    emit_mm2_wave(1, w, bt, widx)
                    widx += 1

    nc.finalize()
    return nc


_NC_CACHE = {}


def _get_nc(use_b=False, use_bias=False):
    key = (use_b, use_bias)
    if key not in _NC_CACHE:
        _NC_CACHE[key] = _build_nc(use_b, use_bias)
    return _NC_CACHE[key]


def _round_fp32r(a):
    """Round fp32 to the fp32r grid (11-bit mantissa; low 12 bits zero)."""
    u = np.ascontiguousarray(a, dtype=np.float32).view(np.uint32)
    r = (u + np.uint32(0x7FF) + ((u >> np.uint32(12)) & np.uint32(1))) & np.uint32(0xFFFFF000)
    return r.view(np.float32)


def build(inputs, context, U, S, V, W, B, bias):
    """Host-side packing: returns (nc, in_maps)."""
    use_b = bool(np.any(np.asarray(B)))
    use_bias = bool(np.any(np.asarray(bias)))

    # U with S folded into its columns, chunked for the stream:
    # u8[q, p, j*RANK + r] = (U*S)[(q*KPQ+j)*128 + p, r]
    US = (np.asarray(U, np.float32) * np.asarray(S, np.float32)[None, :]).astype(bf16)
    u8 = np.ascontiguousarray(
        US.reshape(NQ, KPQ, P, RANK).transpose(0, 2, 1, 3).reshape(NQ, P, KPQ * RANK)
    )

    # VT pieces: vt4[c, p, rt*WU + m'] = V.T[rt*128 + p, c*WU + m']
    VTb = np.asarray(V, np.float32).T.astype(bf16)          # [RANK, UNITS]
    vt4 = np.ascontiguousarray(
        VTb.reshape(RT, P, NW, WU).transpose(2, 1, 0, 3).reshape(NW, P, RT * WU)
    )

    Wk = np.asarray(W, np.float32).astype(bf16).reshape(KC_CTX, P, RANK)
    ctxT = np.asarray(context, np.float32).astype(bf16).T   # [N_CTX, B_SZ]
    xT = np.asarray(inputs, np.float32).astype(bf16).T      # [N_IN, B_SZ]

    bvec = np.ascontiguousarray(np.asarray(B, np.float32).reshape(RT, P).T)
    brow = np.empty((1, P + UNITS), np.float32)
    brow[0, :P] = 1.0
    brow[0, P:] = 2.0 * np.asarray(bias, np.float32)
    brow = _round_fp32r(brow)

    in_maps = []
    for c in range(N_CORES):
        sl = slice(c * BS, (c + 1) * BS)
        # wctx[p, k*(RANK+BS) + ...] = [W_k | ctx_k] per contraction chunk
        wc = np.empty((KC_CTX, P, RANK + BS), bf16)
        wc[:, :, :RANK] = Wk
        wc[:, :, RANK:] = ctxT[:, sl].reshape(KC_CTX, P, BS)
        wctx = np.ascontiguousarray(
            wc.transpose(1, 0, 2).reshape(P, KC_CTX * (RANK + BS))
        )
        # xg[s, g, p, j*BSB + b'] = xT[(g*KPQ+j)*128 + p, c*BS + s*BSB + b']
        xc = xT[:, sl]                                       # [N_IN, BS]
        xgc = np.ascontiguousarray(
            xc.reshape(NQ, KPQ, P, NSB, BSB)
              .transpose(3, 0, 2, 1, 4)
              .reshape(NSB, NQ, P, KPQ * BSB)
        )
        m = {"wctx": wctx, "u8": u8, "xg": xgc, "vt4": vt4}
        if use_b:
            m["bvec"] = bvec
        if use_bias:
            m["brow"] = brow
        in_maps.append(m)
    return _get_nc(use_b, use_bias), in_maps


def gather_out(results):
    out = np.empty((B_SZ, UNITS), dtype=np.float32)
    for c in range(N_CORES):
        out[c * BS:(c + 1) * BS, :] = results[c]["out_d"].astype(np.float32)
    return out


def kernel(inputs, context, U, S, V, W, B, bias):
    nc, in_maps = build(inputs, context, U, S, V, W, B, bias)
    res = run_bass_kernel_spmd(nc, in_maps, list(range(N_CORES)))
    return gather_out(res.results)


# revision 7
# speedup vs baseline: 1.0447x; 1.0447x over previous
"""CASVDDenseMul fused kernel for 8 Trainium2 NeuronCores.

Reference computation (fp32):
    chi = sigmoid(context @ W + B)          # [B, R]
    t   = (inputs @ U) * (S * chi)          # [B, R]
    out = relu(t @ V.T + 2*bias)            # [B, UNITS]

Sharding: data-parallel over batch; each of the 8 cores handles 512 rows.
All factor weights (U, S, V, W, B, bias) are replicated.

Design notes (v2 -- wire-saturation-first):
  - Everything travels as bf16 (PE runs one element/cell/cycle for any
    dtype; rel-err ~3.5e-3 vs the 2e-2 gate; fp8 anywhere measured over
    the gate). S folded into U's columns on the host. 13.4MB/core total
    HBM traffic = ~35us of wire at the ~383GB/s the two HWDGE rings
    sustain; PE work is ~29us -- the kernel is wire-bound, so the whole
    game is keeping the HBM wire busy 100% of the span.
  - v1 streamed x fully before VT and ran all of mm2 after the stream:
    the wire idled 5.4us (all inputs landed, no outputs ready yet) and
    mm2's 13.8us ran serial after it.  v2 reorders the stream
    [U+xA | wctx | VT | xB] and the PE program
    [chi, mm1-A, t'A, mm2-A, mm1-B, t'B, mm2-B]: mm2 on batch
    sub-block A runs inside xB's streaming window, so after the last
    input byte only mm1-B's tail + mm2-B remain on the PE while the
    16 output waves (staged in SBUF, queued FIFO behind the inputs on
    the same two rings) drain the wire without a gap.
  - Ring order == consumption order, ~4.6MB per ring balanced so both
    rings finish inputs together (outputs on a ring drain strictly
    after that ring's inputs -- HWDGE rings are FIFO -- so early waves
    can't steal bandwidth from the xB stream the PE is waiting on).
  - mm2 waves: two single-bank [128,512] PSUM tiles per 1024-unit wave
    from a 5-deep pool (bank reuse ~2.5 waves behind the matmuls,
    beyond the drain->evac->reuse latency chain). 512-wide matmuls,
    rank pairs share one LDWEIGHTS, evacuation split ACT(h0)/DVE(h1).
  - PSUM note: start=True clears has_written BANK-wide, so only the
    first matmul into a shared bank carries start=True.
"""

import numpy as np
import ml_dtypes

from concourse import bacc, mybir
from concourse import tile
from concourse.bass_utils import run_bass_kernel_spmd

N_CORES = 8
B_SZ, N_IN, N_CTX, UNITS, RANK = 4096, 4096, 512, 4096, 256
BS = B_SZ // N_CORES   # 512 batch rows per core

P = 128
KC_IN = N_IN // P      # 32 contraction chunks for x @ U
KC_CTX = N_CTX // P    # 4  contraction chunks for ctx @ W
RT = RANK // P         # 2  rank tiles
NQ = 4                 # U/x stream pieces
KPQ = KC_IN // NQ      # 8 chunks per piece
NSB = 2                # batch sub-blocks
BSB = BS // NSB        # 256 batch cols per sub-block
NBT = BSB // P         # 2 batch tiles (128) per sub-block
NW = 4                 # VT pieces / unit waves (1024 units each)
WU = UNITS // NW       # 1024 units per wave

BF16 = mybir.dt.bfloat16
FP32 = mybir.dt.float32
FP32R = mybir.dt.float32r

bf16 = ml_dtypes.bfloat16


def _build_nc(use_b, use_bias):
    nc = bacc.Bacc("TRN2", target_bir_lowering=False, debug=False, enable_asserts=False)

    wctx = nc.declare_dram_parameter("wctx", [P, KC_CTX * (RANK + BS)], BF16, isOutput=False)
    u8 = nc.declare_dram_parameter("u8", [NQ, P, KPQ * RANK], BF16, isOutput=False)
    xg = nc.declare_dram_parameter("xg", [NSB, NQ, P, KPQ * BSB], BF16, isOutput=False)
    vt4 = nc.declare_dram_parameter("vt4", [NW, P, RT * WU], BF16, isOutput=False)
    if use_b:
        bvec = nc.declare_dram_parameter("bvec", [P, RT], FP32, isOutput=False)
    if use_bias:
        brow = nc.declare_dram_parameter("brow", [1, P + UNITS], FP32R, isOutput=False)
    out_d = nc.declare_dram_parameter("out_d", [BS, UNITS], BF16, isOutput=True)

    with tile.TileContext(nc) as tc:
        with (
            tc.tile_pool(name="small", bufs=1) as small,
            tc.tile_pool(name="stream", bufs=1) as stream,
            tc.tile_pool(name="acts", bufs=1) as acts,
            tc.tile_pool(name="ostage", bufs=16) as ostage,
            tc.tile_pool(name="pt", bufs=1, space="PSUM") as pt,
            tc.tile_pool(name="pout", bufs=6, space="PSUM") as pout,
        ):
            # ---- SBUF tiles ----
            wctx_sb = small.tile([P, KC_CTX * (RANK + BS)], BF16, tag="wctx")
            u_sb = small.tile([P, NQ, KPQ * RANK], BF16, tag="u")
            x_sb = [[stream.tile([P, KPQ * BSB], BF16, tag=f"x{s}{g}", name=f"x{s}{g}")
                     for g in range(NQ)] for s in range(NSB)]
            vt_sb = small.tile([P, NW, RT * WU], BF16, tag="vt")
            if use_b:
                bvec_sb = small.tile([P, RT], FP32, tag="bvec")
            if use_bias:
                brow_sb = small.tile([1, P + UNITS], FP32R, tag="brow")
            s_chi = acts.tile([P, RT, BS], FP32, tag="schi")
            t_sb = [acts.tile([P, RT, BSB], BF16, tag=f"tsb{s}", name=f"tsb{s}")
                    for s in range(NSB)]
            junk = acts.tile([P, P], BF16, tag="junk")

            # ---- DMA issue queues.  Per-ring order == consumption
            # order; the two HWDGE rings are FIFO, so placing VT before
            # xB and all outputs after all inputs enforces the phase
            # plan at the wire level.  ~4.6MB per ring, balanced.
            HB = (KC_CTX // 2) * (RANK + BS)   # wctx half (cols)
            nc.sync.dma_start(u_sb[:, 0, :], u8[0])                    # A1
            nc.scalar.dma_start(x_sb[0][0][:], xg[0, 0])               # B1
            nc.sync.dma_start(x_sb[0][1][:], xg[0, 1])                 # A2
            nc.scalar.dma_start(wctx_sb[:, HB:], wctx[:, HB:])         # B2
            if use_b:
                nc.sync.dma_start(bvec_sb[:], bvec[:])
            if use_bias:
                nc.scalar.dma_start(brow_sb[:], brow[:])
            nc.sync.dma_start(wctx_sb[:, :HB], wctx[:, :HB])           # A3
            nc.scalar.dma_start(u_sb[:, 1, :], u8[1])                  # B3
            nc.sync.dma_start(u_sb[:, 2, :], u8[2])                    # A4
            nc.scalar.dma_start(x_sb[0][2][:], xg[0, 2])               # B4
            nc.sync.dma_start(x_sb[0][3][:], xg[0, 3])                 # A5
            nc.scalar.dma_start(u_sb[:, 3, :], u8[3])                  # B5
            nc.sync.dma_start(vt_sb[:, 0, :], vt4[0])                  # A6
            nc.sync.dma_start(vt_sb[:, 2, :], vt4[2])                  # A7
            nc.sync.dma_start(x_sb[1][0][:], xg[1, 0])                 # A8
            nc.sync.dma_start(x_sb[1][2][:], xg[1, 2])                 # A9
            # Scalar's remaining issues (B6-B9) are emitted AFTER the chi
            # block below: scalar is also the sigmoid engine, and with 9
            # issues ahead of them the sigmoids sit behind the semaphore-
            # recycle chain (issue #9 waits on #5's data, ~21.6us) --
            # which pushed t' and all of mm2-A past the xB window.  With
            # only 5 issues ahead, the sigmoids run as soon as the chi
            # matmuls finish (~19.5us), keeping mm2-A off the critical
            # path in both Tile's scheduling sim and on hardware.

            psum_t = [pt.tile([P, RT * BSB], FP32, tag=f"pt{s}", name=f"pt{s}")
                      for s in range(NSB)]

            # ---- PE warm-up: keep the HAM activity window busy from t=0
            # so the clock gate lifts to 2.4 GHz before the real stream.
            # Targets psum_t[0], whose contents mm1-A's start=True clears.
            nc.gpsimd.memset(junk[:], 0.0)
            for _ in range(16):
                nc.tensor.matmul(
                    psum_t[0][:, :P], junk[:], junk[:],
                    start=True, stop=True, skip_group_check=True,
                )

            # ---- chi' = sigmoid(W.T @ ctxT + B)  (S folded into U) ----
            # chi psum tiles come from the pout pool (same [P,512] fp32
            # shape): chi-rt0/rt1 get their OWN banks, so the two matmul
            # groups + sigmoids pipeline instead of serializing on one
            # bank -- keeping chi (and thus t' and mm2-A) off the
            # critical path.  The banks recycle into the wave rotation.
            for rt in range(RT):
                psum_chi = pout.tile([P, BS], FP32, tag="po", name=f"pchi{rt}")
                for k in range(KC_CTX):
                    base = k * (RANK + BS)
                    nc.tensor.matmul(
                        psum_chi[:],
                        wctx_sb[:, base + rt * P: base + (rt + 1) * P],
                        wctx_sb[:, base + RANK: base + RANK + BS],
                        start=(k == 0), stop=(k == KC_CTX - 1),
                        skip_group_check=True,
                    )
                nc.scalar.activation(
                    s_chi[:, rt, :], psum_chi[:],
                    mybir.ActivationFunctionType.Sigmoid,
                    bias=(bvec_sb[:, rt:rt + 1] if use_b else 0.0), scale=1.0,
                )

            # Scalar ring, second half (see note at the issue queue above).
            nc.scalar.dma_start(vt_sb[:, 1, :], vt4[1])                # B6
            nc.scalar.dma_start(vt_sb[:, 3, :], vt4[3])                # B7
            nc.scalar.dma_start(x_sb[1][1][:], xg[1, 1])               # B8
            nc.scalar.dma_start(x_sb[1][3][:], xg[1, 3])               # B9

            def emit_mm1_piece(s, q):
                # psum_t[s][:, rt*BSB:(rt+1)*BSB] += U'_k.T @ x_k, k in piece q.
                # Both rank-half groups share ONE psum bank; start=True clears
                # has_written BANK-wide, so only the very first matmul carries
                # it (the rt1 group's k=0 lands on cleared bits and start=False
                # already overwrites).
                for j in range(KPQ):
                    k = q * KPQ + j
                    for rt in range(RT):
                        nc.tensor.matmul(
                            psum_t[s][:, rt * BSB:(rt + 1) * BSB],
                            u_sb[:, q, j * RANK + rt * P: j * RANK + (rt + 1) * P],
                            x_sb[s][q][:, j * BSB:(j + 1) * BSB],
                            start=(k == 0 and rt == 0),
                            stop=(k == KC_IN - 1),
                            skip_group_check=True,
                        )

            def emit_tprime(s):
                for rt in range(RT):
                    nc.vector.tensor_mul(
                        t_sb[s][:, rt, :],
                        psum_t[s][:, rt * BSB:(rt + 1) * BSB],
                        s_chi[:, rt, s * BSB:(s + 1) * BSB],
                    )

            def emit_mm2_wave(s, w, bt, widx):
                # out[bt-rows, wave-units] = t'.T @ VT (+ 2*bias), relu, DMA.
                pw = [pout.tile([P, 512], FP32, tag="po", name=f"po{widx}h{h}")
                      for h in range(2)]
                for rt in range(RT):
                    for h in range(2):
                        nc.tensor.matmul(
                            pw[h][:],
                            t_sb[s][:, rt, bt * P:(bt + 1) * P],
                            vt_sb[:, w, rt * WU + h * 512: rt * WU + (h + 1) * 512],
                            start=(rt == 0),
                            stop=(rt == RT - 1 and not use_bias),
                            skip_group_check=True,
                        )
                if use_bias:
                    for h in range(2):
                        nc.tensor.matmul(
                            pw[h][:],
                            brow_sb[:, 0:P],
                            brow_sb[:, P + w * WU + h * 512: P + w * WU + (h + 1) * 512],
                            start=False, stop=True,
                            skip_group_check=True,
                        )
                o_sb = ostage.tile([P, WU], BF16, tag="osb")
                nc.scalar.activation(
                    o_sb[:, 0:512], pw[0][:],
                    mybir.ActivationFunctionType.Relu,
                )
                nc.vector.tensor_scalar(
                    o_sb[:, 512:WU], pw[1][:], 0.0, None,
                    op0=mybir.AluOpType.max,
                )
                rows = slice(s * BSB + bt * P, s * BSB + (bt + 1) * P)
                cols = slice(w * WU, (w + 1) * WU)
                eng = nc.scalar if widx % 2 == 0 else nc.sync
                eng.dma_start(out_d[rows, cols], o_sb[:])

            # PE program order: mm1-A rides the U/xA stream, mm2-A rides
            # VT's arrival (inside xB's streaming window), mm1-B's tail +
            # mm2-B are all that remain after the last input byte.
            widx = 0
            for q in range(NQ):
                emit_mm1_piece(0, q)
            emit_tprime(0)
            for w in range(NW):
                for bt in range(NBT):
                    emit_mm2_wave(0, w, bt, widx)
                    widx += 1
            for q in range(NQ):
                emit_mm1_piece(1, q)
            emit_tprime(1)
            for w in range(NW):
                for bt in range(NBT):
                    emit_mm2_wave(1, w, bt, widx)
                    widx += 1

    nc.finalize()
    return nc


_NC_CACHE = {}


def _get_nc(use_b=False, use_bias=False):
    key = (use_b, use_bias)
    if key not in _NC_CACHE:
        _NC_CACHE[key] = _build_nc(use_b, use_bias)
    return _NC_CACHE[key]


def _round_fp32r(a):
    """Round fp32 to the fp32r grid (11-bit mantissa; low 12 bits zero)."""
    u = np.ascontiguousarray(a, dtype=np.float32).view(np.uint32)
    r = (u + np.uint32(0x7FF) + ((u >> np.uint32(12)) & np.uint32(1))) & np.uint32(0xFFFFF000)
    return r.view(np.float32)


def build(inputs, context, U, S, V, W, B, bias):
    """Host-side packing: returns (nc, in_maps)."""
    use_b = bool(np.any(np.asarray(B)))
    use_bias = bool(np.any(np.asarray(bias)))

    # U with S folded into its columns, chunked for the stream:
    # u8[q, p, j*RANK + r] = (U*S)[(q*KPQ+j)*128 + p, r]
    US = (np.asarray(U, np.float32) * np.asarray(S, np.float32)[None, :]).astype(bf16)
    u8 = np.ascontiguousarray(
        US.reshape(NQ, KPQ, P, RANK).transpose(0, 2, 1, 3).reshape(NQ, P, KPQ * RANK)
    )

    # VT pieces: vt4[c, p, rt*WU + m'] = V.T[rt*128 + p, c*WU + m']
    VTb = np.asarray(V, np.float32).T.astype(bf16)          # [RANK, UNITS]
    vt4 = np.ascontiguousarray(
        VTb.reshape(RT, P, NW, WU).transpose(2, 1, 0, 3).reshape(NW, P, RT * WU)
    )

    Wk = np.asarray(W, np.float32).astype(bf16).reshape(KC_CTX, P, RANK)
    ctxT = np.asarray(context, np.float32).astype(bf16).T   # [N_CTX, B_SZ]
    xT = np.asarray(inputs, np.float32).astype(bf16).T      # [N_IN, B_SZ]

    bvec = np.ascontiguousarray(np.asarray(B, np.float32).reshape(RT, P).T)
    brow = np.empty((1, P + UNITS), np.float32)
    brow[0, :P] = 1.0
    brow[0, P:] = 2.0 * np.asarray(bias, np.float32)
    brow = _round_fp32r(brow)

    in_maps = []
    for c in range(N_CORES):
        sl = slice(c * BS, (c + 1) * BS)
        # wctx[p, k*(RANK+BS) + ...] = [W_k | ctx_k] per contraction chunk
        wc = np.empty((KC_CTX, P, RANK + BS), bf16)
        wc[:, :, :RANK] = Wk
        wc[:, :, RANK:] = ctxT[:, sl].reshape(KC_CTX, P, BS)
        wctx = np.ascontiguousarray(
            wc.transpose(1, 0, 2).reshape(P, KC_CTX * (RANK + BS))
        )
        # xg[s, g, p, j*BSB + b'] = xT[(g*KPQ+j)*128 + p, c*BS + s*BSB + b']
        xc = xT[:, sl]                                       # [N_IN, BS]
        xgc = np.ascontiguousarray(
            xc.reshape(NQ, KPQ, P, NSB, BSB)
              .transpose(3, 0, 2, 1, 4)
              .reshape(NSB, NQ, P, KPQ * BSB)
        )
        m = {"wctx": wctx, "u8": u8, "xg": xgc, "vt4": vt4}
        if use_b:
            m["bvec"] = bvec
        if use_bias:
            m["brow"] = brow
        in_maps.append(m)
    return _get_nc(use_b, use_bias), in_maps


def gather_out(results):
    out = np.empty((B_SZ, UNITS), dtype=np.float32)
    for c in range(N_CORES):
        out[c * BS:(c + 1) * BS, :] = results[c]["out_d"].astype(np.float32)
    return out


def kernel(inputs, context, U, S, V, W, B, bias):
    nc, in_maps = build(inputs, context, U, S, V, W, B, bias)
    res = run_bass_kernel_spmd(nc, in_maps, list(range(N_CORES)))
    return gather_out(res.results)


# revision 10
# speedup vs baseline: 1.1110x; 1.0635x over previous
"""CASVDDenseMul fused kernel for 8 Trainium2 NeuronCores.

Reference computation (fp32):
    chi = sigmoid(context @ W + B)          # [B, R]
    t   = (inputs @ U) * (S * chi)          # [B, R]
    out = relu(t @ V.T + 2*bias)            # [B, UNITS]

Sharding: data-parallel over batch; each of the 8 cores handles 512 rows.
All factor weights (U, S, V, W, B, bias) are replicated.

Design notes (v2 -- wire-saturation-first):
  - Everything travels as bf16 (PE runs one element/cell/cycle for any
    dtype; rel-err ~3.5e-3 vs the 2e-2 gate; fp8 anywhere measured over
    the gate). S folded into U's columns on the host. 13.4MB/core total
    HBM traffic = ~35us of wire at the ~383GB/s the two HWDGE rings
    sustain; PE work is ~29us -- the kernel is wire-bound, so the whole
    game is keeping the HBM wire busy 100% of the span.
  - v1 streamed x fully before VT and ran all of mm2 after the stream:
    the wire idled 5.4us (all inputs landed, no outputs ready yet) and
    mm2's 13.8us ran serial after it.  v2 reorders the stream
    [U+xA | wctx | VT | xB] and the PE program
    [chi, mm1-A, t'A, mm2-A, mm1-B, t'B, mm2-B]: mm2 on batch
    sub-block A runs inside xB's streaming window, so after the last
    input byte only mm1-B's tail + mm2-B remain on the PE while the
    16 output waves (staged in SBUF, queued FIFO behind the inputs on
    the same two rings) drain the wire without a gap.
  - Ring order == consumption order, ~4.6MB per ring balanced so both
    rings finish inputs together (outputs on a ring drain strictly
    after that ring's inputs -- HWDGE rings are FIFO -- so early waves
    can't steal bandwidth from the xB stream the PE is waiting on).
  - mm2 waves: two single-bank [128,512] PSUM tiles per 1024-unit wave
    from a 5-deep pool (bank reuse ~2.5 waves behind the matmuls,
    beyond the drain->evac->reuse latency chain). 512-wide matmuls,
    rank pairs share one LDWEIGHTS, evacuation split ACT(h0)/DVE(h1).
  - PSUM note: start=True clears has_written BANK-wide, so only the
    first matmul into a shared bank carries start=True.
"""

import numpy as np
import ml_dtypes

from concourse import bacc, mybir
from concourse import tile
from concourse.bass_utils import run_bass_kernel_spmd

N_CORES = 8
B_SZ, N_IN, N_CTX, UNITS, RANK = 4096, 4096, 512, 4096, 256
BS = B_SZ // N_CORES   # 512 batch rows per core

P = 128
KC_IN = N_IN // P      # 32 contraction chunks for x @ U
KC_CTX = N_CTX // P    # 4  contraction chunks for ctx @ W
RT = RANK // P         # 2  rank tiles
NQ = 4                 # U/x stream pieces
KPQ = KC_IN // NQ      # 8 chunks per piece
NSB = 2                # batch sub-blocks
BSB = BS // NSB        # 256 batch cols per sub-block
NBT = BSB // P         # 2 batch tiles (128) per sub-block
NW = 4                 # VT pieces / unit waves (1024 units each)
WU = UNITS // NW       # 1024 units per wave

BF16 = mybir.dt.bfloat16
FP32 = mybir.dt.float32
FP32R = mybir.dt.float32r

bf16 = ml_dtypes.bfloat16


def _build_nc(use_b, use_bias):
    nc = bacc.Bacc("TRN2", target_bir_lowering=False, debug=False, enable_asserts=False)

    wctx = nc.declare_dram_parameter("wctx", [P, KC_CTX * (RANK + BS)], BF16, isOutput=False)
    u8 = nc.declare_dram_parameter("u8", [NQ, P, KPQ * RANK], BF16, isOutput=False)
    xg = nc.declare_dram_parameter("xg", [NSB, NQ, P, KPQ * BSB], BF16, isOutput=False)
    vt4 = nc.declare_dram_parameter("vt4", [NW, P, RT * WU], BF16, isOutput=False)
    if use_b:
        bvec = nc.declare_dram_parameter("bvec", [P, RT], FP32, isOutput=False)
    if use_bias:
        brow = nc.declare_dram_parameter("brow", [1, P + UNITS], FP32R, isOutput=False)
    out_d = nc.declare_dram_parameter("out_d", [BS, UNITS], BF16, isOutput=True)

    with tile.TileContext(nc) as tc:
        with (
            tc.tile_pool(name="small", bufs=1) as small,
            tc.tile_pool(name="stream", bufs=1) as stream,
            tc.tile_pool(name="acts", bufs=1) as acts,
            tc.tile_pool(name="ostage", bufs=16) as ostage,
            tc.tile_pool(name="pt", bufs=1, space="PSUM") as pt,
            tc.tile_pool(name="pout", bufs=6, space="PSUM") as pout,
        ):
            # ---- SBUF tiles ----
            wctx_sb = small.tile([P, KC_CTX * (RANK + BS)], BF16, tag="wctx")
            u_sb = small.tile([P, NQ, KPQ * RANK], BF16, tag="u")
            x_sb = [[stream.tile([P, KPQ * BSB], BF16, tag=f"x{s}{g}", name=f"x{s}{g}")
                     for g in range(NQ)] for s in range(NSB)]
            vt_sb = small.tile([P, NW, RT * WU], BF16, tag="vt")
            if use_b:
                bvec_sb = small.tile([P, RT], FP32, tag="bvec")
            if use_bias:
                brow_sb = small.tile([1, P + UNITS], FP32R, tag="brow")
            s_chi = acts.tile([P, RT, BS], FP32, tag="schi")
            t_sb = [acts.tile([P, RT, BSB], BF16, tag=f"tsb{s}", name=f"tsb{s}")
                    for s in range(NSB)]
            junk = acts.tile([P, P], BF16, tag="junk")

            # ---- DMA issue queue.  ONE HWDGE ring (sync) carries every
            # transfer, in exact consumption order: a single queue
            # sustains the full ~390GB/s HBM rate solo, and single-queue
            # FIFO gives a TOTAL order on the wire -- no cross-ring
            # round-robin (which let early output waves steal bandwidth
            # from the xB input tail), no issue-skew between rings, and
            # no coupling with the scalar engine's sigmoid/relu work.
            nc.sync.dma_start(u_sb[:, 0, :], u8[0])
            nc.sync.dma_start(x_sb[0][0][:], xg[0, 0])
            nc.sync.dma_start(u_sb[:, 1, :], u8[1])
            nc.sync.dma_start(x_sb[0][1][:], xg[0, 1])
            if use_b:
                nc.sync.dma_start(bvec_sb[:], bvec[:])
            if use_bias:
                nc.sync.dma_start(brow_sb[:], brow[:])
            nc.sync.dma_start(wctx_sb[:], wctx[:])
            nc.sync.dma_start(u_sb[:, 2, :], u8[2])
            nc.sync.dma_start(x_sb[0][2][:], xg[0, 2])
            nc.sync.dma_start(u_sb[:, 3, :], u8[3])
            nc.sync.dma_start(x_sb[0][3][:], xg[0, 3])
            for w in range(NW):
                nc.sync.dma_start(vt_sb[:, w, :], vt4[w])
            for g in range(NQ):
                nc.sync.dma_start(x_sb[1][g][:], xg[1, g])

            psum_t = [pt.tile([P, RT * BSB], FP32, tag=f"pt{s}", name=f"pt{s}")
                      for s in range(NSB)]

            # ---- PE warm-up: keep the HAM activity window busy from t=0
            # so the clock gate lifts to 2.4 GHz before the real stream.
            # Targets psum_t[0], whose contents mm1-A's start=True clears.
            nc.gpsimd.memset(junk[:], 0.0)
            for _ in range(16):
                nc.tensor.matmul(
                    psum_t[0][:, :P], junk[:], junk[:],
                    start=True, stop=True, skip_group_check=True,
                )

            # ---- chi' = sigmoid(W.T @ ctxT + B)  (S folded into U) ----
            # chi psum tiles come from the pout pool (same [P,512] fp32
            # shape): chi-rt0/rt1 get their OWN banks, so the two matmul
            # groups + sigmoids pipeline instead of serializing on one
            # bank -- keeping chi (and thus t' and mm2-A) off the
            # critical path.  The banks recycle into the wave rotation.
            for rt in range(RT):
                psum_chi = pout.tile([P, BS], FP32, tag="po", name=f"pchi{rt}")
                for k in range(KC_CTX):
                    base = k * (RANK + BS)
                    nc.tensor.matmul(
                        psum_chi[:],
                        wctx_sb[:, base + rt * P: base + (rt + 1) * P],
                        wctx_sb[:, base + RANK: base + RANK + BS],
                        start=(k == 0), stop=(k == KC_CTX - 1),
                        skip_group_check=True,
                    )
                nc.scalar.activation(
                    s_chi[:, rt, :], psum_chi[:],
                    mybir.ActivationFunctionType.Sigmoid,
                    bias=(bvec_sb[:, rt:rt + 1] if use_b else 0.0), scale=1.0,
                )

            def emit_mm1_piece(s, q):
                # psum_t[s][:, rt*BSB:(rt+1)*BSB] += U'_k.T @ x_k, k in piece q.
                # Both rank-half groups share ONE psum bank; start=True clears
                # has_written BANK-wide, so only the very first matmul carries
                # it (the rt1 group's k=0 lands on cleared bits and start=False
                # already overwrites).
                for j in range(KPQ):
                    k = q * KPQ + j
                    for rt in range(RT):
                        nc.tensor.matmul(
                            psum_t[s][:, rt * BSB:(rt + 1) * BSB],
                            u_sb[:, q, j * RANK + rt * P: j * RANK + (rt + 1) * P],
                            x_sb[s][q][:, j * BSB:(j + 1) * BSB],
                            start=(k == 0 and rt == 0),
                            stop=(k == KC_IN - 1),
                            skip_group_check=True,
                        )

            def emit_tprime(s):
                for rt in range(RT):
                    nc.vector.tensor_mul(
                        t_sb[s][:, rt, :],
                        psum_t[s][:, rt * BSB:(rt + 1) * BSB],
                        s_chi[:, rt, s * BSB:(s + 1) * BSB],
                    )

            def emit_mm2_wave(s, w, bt, widx):
                # out[bt-rows, wave-units] = t'.T @ VT (+ 2*bias), relu, DMA.
                pw = [pout.tile([P, 512], FP32, tag="po", name=f"po{widx}h{h}")
                      for h in range(2)]
                for rt in range(RT):
                    for h in range(2):
                        nc.tensor.matmul(
                            pw[h][:],
                            t_sb[s][:, rt, bt * P:(bt + 1) * P],
                            vt_sb[:, w, rt * WU + h * 512: rt * WU + (h + 1) * 512],
                            start=(rt == 0),
                            stop=(rt == RT - 1 and not use_bias),
                            skip_group_check=True,
                        )
                if use_bias:
                    for h in range(2):
                        nc.tensor.matmul(
                            pw[h][:],
                            brow_sb[:, 0:P],
                            brow_sb[:, P + w * WU + h * 512: P + w * WU + (h + 1) * 512],
                            start=False, stop=True,
                            skip_group_check=True,
                        )
                o_sb = ostage.tile([P, WU], BF16, tag="osb")
                nc.scalar.activation(
                    o_sb[:, 0:512], pw[0][:],
                    mybir.ActivationFunctionType.Relu,
                )
                nc.vector.tensor_scalar(
                    o_sb[:, 512:WU], pw[1][:], 0.0, None,
                    op0=mybir.AluOpType.max,
                )
                rows = slice(s * BSB + bt * P, s * BSB + (bt + 1) * P)
                cols = slice(w * WU, (w + 1) * WU)
                nc.sync.dma_start(out_d[rows, cols], o_sb[:])

            # PE program order: mm1-A rides the U/xA stream, mm2-A rides
            # VT's arrival (inside xB's streaming window), mm1-B's tail +
            # mm2-B are all that remain after the last input byte.
            widx = 0
            for q in range(NQ):
                emit_mm1_piece(0, q)
            emit_tprime(0)
            for w in range(NW):
                for bt in range(NBT):
                    emit_mm2_wave(0, w, bt, widx)
                    widx += 1
            for q in range(NQ):
                emit_mm1_piece(1, q)
            emit_tprime(1)
            for w in range(NW):
                for bt in range(NBT):
                    emit_mm2_wave(1, w, bt, widx)
                    widx += 1

    nc.finalize()
    return nc


_NC_CACHE = {}


def _get_nc(use_b=False, use_bias=False):
    key = (use_b, use_bias)
    if key not in _NC_CACHE:
        _NC_CACHE[key] = _build_nc(use_b, use_bias)
    return _NC_CACHE[key]


def _round_fp32r(a):
    """Round fp32 to the fp32r grid (11-bit mantissa; low 12 bits zero)."""
    u = np.ascontiguousarray(a, dtype=np.float32).view(np.uint32)
    r = (u + np.uint32(0x7FF) + ((u >> np.uint32(12)) & np.uint32(1))) & np.uint32(0xFFFFF000)
    return r.view(np.float32)


def build(inputs, context, U, S, V, W, B, bias):
    """Host-side packing: returns (nc, in_maps)."""
    use_b = bool(np.any(np.asarray(B)))
    use_bias = bool(np.any(np.asarray(bias)))

    # U with S folded into its columns, chunked for the stream:
    # u8[q, p, j*RANK + r] = (U*S)[(q*KPQ+j)*128 + p, r]
    US = (np.asarray(U, np.float32) * np.asarray(S, np.float32)[None, :]).astype(bf16)
    u8 = np.ascontiguousarray(
        US.reshape(NQ, KPQ, P, RANK).transpose(0, 2, 1, 3).reshape(NQ, P, KPQ * RANK)
    )

    # VT pieces: vt4[c, p, rt*WU + m'] = V.T[rt*128 + p, c*WU + m']
    VTb = np.asarray(V, np.float32).T.astype(bf16)          # [RANK, UNITS]
    vt4 = np.ascontiguousarray(
        VTb.reshape(RT, P, NW, WU).transpose(2, 1, 0, 3).reshape(NW, P, RT * WU)
    )

    Wk = np.asarray(W, np.float32).astype(bf16).reshape(KC_CTX, P, RANK)
    ctxT = np.asarray(context, np.float32).astype(bf16).T   # [N_CTX, B_SZ]
    xT = np.asarray(inputs, np.float32).astype(bf16).T      # [N_IN, B_SZ]

    bvec = np.ascontiguousarray(np.asarray(B, np.float32).reshape(RT, P).T)
    brow = np.empty((1, P + UNITS), np.float32)
    brow[0, :P] = 1.0
    brow[0, P:] = 2.0 * np.asarray(bias, np.float32)
    brow = _round_fp32r(brow)

    in_maps = []
    for c in range(N_CORES):
        sl = slice(c * BS, (c + 1) * BS)
        # wctx[p, k*(RANK+BS) + ...] = [W_k | ctx_k] per contraction chunk
        wc = np.empty((KC_CTX, P, RANK + BS), bf16)
        wc[:, :, :RANK] = Wk
        wc[:, :, RANK:] = ctxT[:, sl].reshape(KC_CTX, P, BS)
        wctx = np.ascontiguousarray(
            wc.transpose(1, 0, 2).reshape(P, KC_CTX * (RANK + BS))
        )
        # xg[s, g, p, j*BSB + b'] = xT[(g*KPQ+j)*128 + p, c*BS + s*BSB + b']
        xc = xT[:, sl]                                       # [N_IN, BS]
        xgc = np.ascontiguousarray(
            xc.reshape(NQ, KPQ, P, NSB, BSB)
              .transpose(3, 0, 2, 1, 4)
              .reshape(NSB, NQ, P, KPQ * BSB)
        )
        m = {"wctx": wctx, "u8": u8, "xg": xgc, "vt4": vt4}
        if use_b:
            m["bvec"] = bvec
        if use_bias:
            m["brow"] = brow
        in_maps.append(m)
    return _get_nc(use_b, use_bias), in_maps


def gather_out(results):
    out = np.empty((B_SZ, UNITS), dtype=np.float32)
    for c in range(N_CORES):
        out[c * BS:(c + 1) * BS, :] = results[c]["out_d"].astype(np.float32)
    return out


def kernel(inputs, context, U, S, V, W, B, bias):
    nc, in_maps = build(inputs, context, U, S, V, W, B, bias)
    res = run_bass_kernel_spmd(nc, in_maps, list(range(N_CORES)))
    return gather_out(res.results)


# revision 16
# speedup vs baseline: 1.1510x; 1.0360x over previous
"""CASVDDenseMul fused kernel for 8 Trainium2 NeuronCores.

Reference computation (fp32):
    chi = sigmoid(context @ W + B)          # [B, R]
    t   = (inputs @ U) * (S * chi)          # [B, R]
    out = relu(t @ V.T + 2*bias)            # [B, UNITS]

Sharding: data-parallel over batch; each of the 8 cores handles 512 rows.
All factor weights (U, S, V, W, B, bias) are replicated.

Design notes (v6 -- single-queue total order, measured-latency driven):
  - All-bf16 transport (fp8 anywhere measured over the 2e-2 gate).
    13.4MB/core HBM traffic; PE work ~29us; wire ~34us => wire-bound.
  - ONE HWDGE ring (sync) carries EVERY transfer in consumption order.
    Measured: a single queue sustains 390-418 GB/s solo -- same as two
    queues combined -- while two queues round-robin at packet
    granularity, which let early output waves steal bandwidth from the
    input tail and let issue-side skew reorder arrivals.  Single-queue
    FIFO gives a total order on the wire by construction: inputs
    [U/x interleaved, wctx, VT], then the 16 output waves.
  - Measured: each DMA->compute dependency pays ~2.5us completion
    latency (HBM write-receipt before the semaphore fires) on top of
    wire time.  Consequences baked in here: (a) no batch sub-blocking
    -- with the PE saturated from t' onward, end = t'_time + remaining
    PE work, and moving any mm1 after t' costs (PE time) more than the
    earlier t' saves (wire time); (b) x streams in 0.52MB pieces and
    vt piece 0 in halves, so the last dependency's receipt overlaps
    the next transfer's wire time; (c) t' is computed in 8 [P,128]
    column blocks emitted in wave-consumption order, so mm2's first
    wave starts ~0.2us after mm1's last accumulation, not ~1.7us.
  - chi runs entirely off the critical path: its two matmul groups get
    their OWN psum banks (from the wave pool, which they recycle into)
    so the groups+sigmoids pipeline, and the scalar engine issues no
    DMAs, so sigmoids can't be trapped behind DMA-issue semaphore
    recycling (both failure modes observed in earlier revisions).
  - mm2 waves: two single-bank [128,512] psum tiles per 1024-unit wave
    from a 6-deep pool; 512-wide matmuls, rank pairs share one
    LDWEIGHTS, evacuation split ACT(h0)/DVE(h1); outputs stage in SBUF
    (16 dedicated buffers) and drain the single ring FIFO behind the
    inputs.
  - PSUM: psum_t [P,1024] fp32 = 2 aligned banks (rt-major), wave pool
    6 banks => 8 total.  start=True clears has_written BANK-wide, so
    only the first matmul into a shared bank carries it.
"""

import numpy as np
import ml_dtypes

from concourse import bacc, mybir
from concourse import tile
from concourse.bass_utils import run_bass_kernel_spmd

N_CORES = 8
B_SZ, N_IN, N_CTX, UNITS, RANK = 4096, 4096, 512, 4096, 256
BS = B_SZ // N_CORES   # 512 batch rows per core

P = 128
KC_IN = N_IN // P      # 32 contraction chunks for x @ U
KC_CTX = N_CTX // P    # 4  contraction chunks for ctx @ W
RT = RANK // P         # 2  rank tiles
NQ = 16                # x stream pieces (0.26MB each)
KPQ = KC_IN // NQ      # 2 chunks per x piece
NU = 4                 # U stream pieces (4 x pieces each)
KPU = KC_IN // NU      # 8 chunks per U piece
NBT = BS // P          # 4 batch tiles
NW = 4                 # VT pieces / unit-wave groups (1024 units each)
WU = UNITS // NW       # 1024 units per wave

BF16 = mybir.dt.bfloat16
FP32 = mybir.dt.float32
FP32R = mybir.dt.float32r

bf16 = ml_dtypes.bfloat16


def _build_nc(use_b, use_bias):
    nc = bacc.Bacc("TRN2", target_bir_lowering=False, debug=False, enable_asserts=False)

    wctx = nc.declare_dram_parameter("wctx", [P, KC_CTX * (RANK + BS)], BF16, isOutput=False)
    u8 = nc.declare_dram_parameter("u8", [NU, P, KPU * RANK], BF16, isOutput=False)
    xg = nc.declare_dram_parameter("xg", [NQ, P, KPQ * BS], BF16, isOutput=False)
    vt4 = nc.declare_dram_parameter("vt4", [NW, P, RT * WU], BF16, isOutput=False)
    if use_b:
        bvec = nc.declare_dram_parameter("bvec", [P, RT], FP32, isOutput=False)
    if use_bias:
        brow = nc.declare_dram_parameter("brow", [1, P + UNITS], FP32R, isOutput=False)
    out_d = nc.declare_dram_parameter("out_d", [BS, UNITS], BF16, isOutput=True)

    with tile.TileContext(nc) as tc:
        with (
            tc.tile_pool(name="small", bufs=1) as small,
            tc.tile_pool(name="stream", bufs=1) as stream,
            tc.tile_pool(name="acts", bufs=1) as acts,
            tc.tile_pool(name="ostage", bufs=16) as ostage,
            tc.tile_pool(name="pt", bufs=1, space="PSUM") as pt,
            tc.tile_pool(name="pout", bufs=6, space="PSUM") as pout,
        ):
            # ---- SBUF tiles ----
            wctx_sb = small.tile([P, KC_CTX * (RANK + BS)], BF16, tag="wctx")
            u_sb = small.tile([P, NU, KPU * RANK], BF16, tag="u")
            x_sb = [stream.tile([P, KPQ * BS], BF16, tag=f"x{g}", name=f"x{g}")
                    for g in range(NQ)]
            vt_sb = small.tile([P, NW, RT * WU], BF16, tag="vt")
            if use_b:
                bvec_sb = small.tile([P, RT], FP32, tag="bvec")
            if use_bias:
                brow_sb = small.tile([1, P + UNITS], FP32R, tag="brow")
            s_chi = acts.tile([P, RT, BS], FP32, tag="schi")
            t_sb = acts.tile([P, RT, BS], BF16, tag="tsb")
            junk = acts.tile([P, P], BF16, tag="junk")

            # ---- DMA issue queue: ONE ring, consumption order.
            # u/x interleave for mm1 in 0.26MB x pieces; wctx mid-stream
            # (chi is a gap-filler); vt0 BEFORE the x tail so its
            # completion receipt retires off the critical path and mm2's
            # first wave is gated only by t'; vt1-3 last (they arrive
            # ahead of the PE's wave consumption); outputs (emitted
            # inside the wave bodies below) drain FIFO after everything.
            nc.sync.dma_start(u_sb[:, 0, :], u8[0])
            for g in range(0, 4):
                nc.sync.dma_start(x_sb[g][:], xg[g])
            if use_b:
                nc.sync.dma_start(bvec_sb[:], bvec[:])
            if use_bias:
                nc.sync.dma_start(brow_sb[:], brow[:])
            nc.sync.dma_start(u_sb[:, 1, :], u8[1])
            for g in range(4, 8):
                nc.sync.dma_start(x_sb[g][:], xg[g])
            nc.sync.dma_start(wctx_sb[:], wctx[:])
            nc.sync.dma_start(vt_sb[:, 0, :], vt4[0])
            nc.sync.dma_start(u_sb[:, 2, :], u8[2])
            for g in range(8, 12):
                nc.sync.dma_start(x_sb[g][:], xg[g])
            nc.sync.dma_start(u_sb[:, 3, :], u8[3])
            for g in range(12, 16):
                nc.sync.dma_start(x_sb[g][:], xg[g])
            for w in range(1, NW):
                nc.sync.dma_start(vt_sb[:, w, :], vt4[w])

            psum_t = pt.tile([P, RT * BS], FP32, tag="pt")

            # ---- PE warm-up: keep the HAM activity window busy from t=0
            # so the clock gate lifts to 2.4 GHz before the real stream.
            # Targets psum_t, whose contents mm1's start=True clears.
            nc.gpsimd.memset(junk[:], 0.0)
            for _ in range(16):
                nc.tensor.matmul(
                    psum_t[:, :P], junk[:], junk[:],
                    start=True, stop=True, skip_group_check=True,
                )

            # ---- chi' = sigmoid(W.T @ ctxT + B)  (S folded into U) ----
            # chi psum tiles come from the wave pool (same [P,512] fp32
            # shape): each matmul group gets its OWN bank, so the groups
            # and sigmoids pipeline instead of serializing; the banks
            # recycle into the wave rotation afterwards.
            for rt in range(RT):
                psum_chi = pout.tile([P, BS], FP32, tag="po", name=f"pchi{rt}")
                for k in range(KC_CTX):
                    base = k * (RANK + BS)
                    nc.tensor.matmul(
                        psum_chi[:],
                        wctx_sb[:, base + rt * P: base + (rt + 1) * P],
                        wctx_sb[:, base + RANK: base + RANK + BS],
                        start=(k == 0), stop=(k == KC_CTX - 1),
                        skip_group_check=True,
                    )
                nc.scalar.activation(
                    s_chi[:, rt, :], psum_chi[:],
                    mybir.ActivationFunctionType.Sigmoid,
                    bias=(bvec_sb[:, rt:rt + 1] if use_b else 0.0), scale=1.0,
                )

            # ---- mm1: psum_t[:, rt*BS:(rt+1)*BS] += U'_k.T @ x_k ----
            # The rt groups live in SEPARATE psum banks of the 2-bank
            # tile, so EACH group's k=0 matmul must carry start=True to
            # clear its own bank's has_written bits (stale from the
            # previous execution otherwise).
            for g in range(NQ):
                for j in range(KPQ):
                    k = g * KPQ + j
                    up = k // KPU             # u piece holding chunk k
                    ju = k - up * KPU         # chunk index within it
                    for rt in range(RT):
                        nc.tensor.matmul(
                            psum_t[:, rt * BS:(rt + 1) * BS],
                            u_sb[:, up, ju * RANK + rt * P: ju * RANK + (rt + 1) * P],
                            x_sb[g][:, j * BS:(j + 1) * BS],
                            start=(k == 0),
                            stop=(k == KC_IN - 1),
                            skip_group_check=True,
                        )

            # ---- t' = psum_t * chi, in [P,128] blocks emitted in wave-
            # consumption order (bt-major) so mm2's first wave needs only
            # the first two blocks, not the whole 1.7us multiply.
            for bt in range(NBT):
                for rt in range(RT):
                    sl = slice(bt * P, (bt + 1) * P)
                    nc.vector.tensor_mul(
                        t_sb[:, rt, sl],
                        psum_t[:, rt * BS + bt * P: rt * BS + (bt + 1) * P],
                        s_chi[:, rt, sl],
                    )

            # ---- mm2 waves: out[bt-rows, wave-units] = t'.T @ VT,
            # (+ 2*bias), relu, stage to SBUF, DMA on the single ring.
            widx = 0
            for w in range(NW):
                for bt in range(NBT):
                    pw = [pout.tile([P, 512], FP32, tag="po", name=f"po{widx}h{h}")
                          for h in range(2)]
                    for rt in range(RT):
                        for h in range(2):
                            nc.tensor.matmul(
                                pw[h][:],
                                t_sb[:, rt, bt * P:(bt + 1) * P],
                                vt_sb[:, w, rt * WU + h * 512: rt * WU + (h + 1) * 512],
                                start=(rt == 0),
                                stop=(rt == RT - 1 and not use_bias),
                                skip_group_check=True,
                            )
                    if use_bias:
                        for h in range(2):
                            nc.tensor.matmul(
                                pw[h][:],
                                brow_sb[:, 0:P],
                                brow_sb[:, P + w * WU + h * 512: P + w * WU + (h + 1) * 512],
                                start=False, stop=True,
                                skip_group_check=True,
                            )
                    o_sb = ostage.tile([P, WU], BF16, tag="osb")
                    nc.scalar.activation(
                        o_sb[:, 0:512], pw[0][:],
                        mybir.ActivationFunctionType.Relu,
                    )
                    nc.vector.tensor_scalar(
                        o_sb[:, 512:WU], pw[1][:], 0.0, None,
                        op0=mybir.AluOpType.max,
                    )
                    rows = slice(bt * P, (bt + 1) * P)
                    cols = slice(w * WU, (w + 1) * WU)
                    if widx == NW * NBT - 1:
                        # Last wave: ship each half as soon as its own
                        # evacuation lands -- trims the final wire tail.
                        nc.sync.dma_start(
                            out_d[rows, w * WU: w * WU + 512], o_sb[:, 0:512])
                        nc.sync.dma_start(
                            out_d[rows, w * WU + 512:(w + 1) * WU], o_sb[:, 512:WU])
                    else:
                        nc.sync.dma_start(out_d[rows, cols], o_sb[:])
                    widx += 1

    nc.finalize()
    return nc


_NC_CACHE = {}


def _get_nc(use_b=False, use_bias=False):
    key = (use_b, use_bias)
    if key not in _NC_CACHE:
        _NC_CACHE[key] = _build_nc(use_b, use_bias)
    return _NC_CACHE[key]


def _round_fp32r(a):
    """Round fp32 to the fp32r grid (11-bit mantissa; low 12 bits zero)."""
    u = np.ascontiguousarray(a, dtype=np.float32).view(np.uint32)
    r = (u + np.uint32(0x7FF) + ((u >> np.uint32(12)) & np.uint32(1))) & np.uint32(0xFFFFF000)
    return r.view(np.float32)


def build(inputs, context, U, S, V, W, B, bias):
    """Host-side packing: returns (nc, in_maps)."""
    use_b = bool(np.any(np.asarray(B)))
    use_bias = bool(np.any(np.asarray(bias)))

    # U with S folded into its columns, chunked for the stream:
    # u8[q, p, j*RANK + r] = (U*S)[(q*KPU+j)*128 + p, r]
    US = (np.asarray(U, np.float32) * np.asarray(S, np.float32)[None, :]).astype(bf16)
    u8 = np.ascontiguousarray(
        US.reshape(NU, KPU, P, RANK).transpose(0, 2, 1, 3).reshape(NU, P, KPU * RANK)
    )

    # VT pieces: vt4[c, p, rt*WU + m'] = V.T[rt*128 + p, c*WU + m']
    VTb = np.asarray(V, np.float32).T.astype(bf16)          # [RANK, UNITS]
    vt4 = np.ascontiguousarray(
        VTb.reshape(RT, P, NW, WU).transpose(2, 1, 0, 3).reshape(NW, P, RT * WU)
    )

    Wk = np.asarray(W, np.float32).astype(bf16).reshape(KC_CTX, P, RANK)
    ctxT = np.asarray(context, np.float32).astype(bf16).T   # [N_CTX, B_SZ]
    xT = np.asarray(inputs, np.float32).astype(bf16).T      # [N_IN, B_SZ]

    bvec = np.ascontiguousarray(np.asarray(B, np.float32).reshape(RT, P).T)
    brow = np.empty((1, P + UNITS), np.float32)
    brow[0, :P] = 1.0
    brow[0, P:] = 2.0 * np.asarray(bias, np.float32)
    brow = _round_fp32r(brow)

    in_maps = []
    for c in range(N_CORES):
        sl = slice(c * BS, (c + 1) * BS)
        # wctx[p, k*(RANK+BS) + ...] = [W_k | ctx_k] per contraction chunk
        wc = np.empty((KC_CTX, P, RANK + BS), bf16)
        wc[:, :, :RANK] = Wk
        wc[:, :, RANK:] = ctxT[:, sl].reshape(KC_CTX, P, BS)
        wctx = np.ascontiguousarray(
            wc.transpose(1, 0, 2).reshape(P, KC_CTX * (RANK + BS))
        )
        # xg[g, p, j*BS + b] = xT[(g*KPQ+j)*128 + p, c*BS + b]
        xc = xT[:, sl]                                       # [N_IN, BS]
        xgc = np.ascontiguousarray(
            xc.reshape(NQ, KPQ, P, BS).transpose(0, 2, 1, 3).reshape(NQ, P, KPQ * BS)
        )
        m = {"wctx": wctx, "u8": u8, "xg": xgc, "vt4": vt4}
        if use_b:
            m["bvec"] = bvec
        if use_bias:
            m["brow"] = brow
        in_maps.append(m)
    return _get_nc(use_b, use_bias), in_maps


def gather_out(results):
    out = np.empty((B_SZ, UNITS), dtype=np.float32)
    for c in range(N_CORES):
        out[c * BS:(c + 1) * BS, :] = results[c]["out_d"].astype(np.float32)
    return out


def kernel(inputs, context, U, S, V, W, B, bias):
    nc, in_maps = build(inputs, context, U, S, V, W, B, bias)
    res = run_bass_kernel_spmd(nc, in_maps, list(range(N_CORES)))
    return gather_out(res.results)
